# revision 1
# baseline (speedup 1.0000x reference)
"""Multi-head attention (B=2, S=2048, D=1024, H=16) on 8 Trainium2 cores.

Sharding: tensor-parallel over heads — 2 heads per core. Each core computes
QKV for its 384 features (contraction over D with the full x), per-head
attention (scores -> exp -> AV, softmax without max-subtraction since scores
are O(1)), and a partial output projection against its 128 rows of Wproj.
The host sums the 8 partial projections and adds bproj.

Engine plan (per core):
  PE   : all matmuls — QKV (bf16 in, f32 psum), scores/AV/proj (f32r).
         V is computed directly in transposed layout (lhsT = x tile block,
         rhs = Wv chunk, N=128 bf16) so no PE transposes are needed.
  ACT  : exp only ([128,1024] per key-chunk, PSUM -> SBUF).
  DVE  : every PSUM drain (q/k/v3/proj), softmax reciprocal, normalize mul.
  SP   : all DMA (loads first in queue order, stores behind them), incl.
         the per-query-reciprocal DRAM-bounce broadcast (measured faster
         on HW than gpsimd partition_broadcast, which the cost model
         underestimates).

Schedule: software-pipelined attention. Per key-chunk kb the PE queue gets
[scores(kb), fillers..., AV(kb-1)] where fillers are QKV-strip or projection
matmuls, so AV never blocks the in-order PE queue while ACT runs exp.
PSUM: scores tag "s" [128,1024]x2 (4 banks) + shared "mm" [128,512]x2
(QKV chains + proj, one open accumulation chain at a time) + two per-head
AV accumulators [65,512] (ones column in v3 yields sumexp in row 64).
"""

import numpy as np

B, S, D, H = 2, 2048, 1024, 16
HD = D // H          # 64
T = B * S            # 4096 tokens
N_CORES = 8
HPC = H // N_CORES   # 2 heads per core

KD = [(i * 128, 128) for i in range(8)]  # contraction chunks over D

_NC_CACHE = {}


def build_nc(reps: int = 1, zero_bias: bool = True, pbcast: str = "bounce",
             skip_exp: bool = False):
    """pbcast: how the per-query softmax reciprocal [1,512] is broadcast
    across 64 partitions — "gpsimd" (partition_broadcast) or "bounce"
    (SBUF->DRAM->SBUF DMA pair; partition-stride-0 SBUF sources are not
    allowed so a direct SBUF->SBUF broadcast DMA is impossible).
    skip_exp: timing-attribution probe — drop the exp (AV reads raw
    scores; results are wrong)."""
    key = (reps, zero_bias, pbcast, skip_exp)
    if key in _NC_CACHE:
        return _NC_CACHE[key]

    from concourse import bacc
    import concourse.bass as bass
    import concourse.mybir as mybir
    import concourse.tile as tile

    F32 = mybir.dt.float32
    F32R = mybir.dt.float32r
    BF16 = mybir.dt.bfloat16
    Exp = mybir.ActivationFunctionType.Exp
    ADD = mybir.AluOpType.add

    nc = bacc.Bacc()
    xT = nc.declare_dram_parameter("xT", [D, T], BF16, isOutput=False)
    wqk = nc.declare_dram_parameter("wqk", [D, 256], BF16, isOutput=False)
    wv = nc.declare_dram_parameter("wv", [D, 128], BF16, isOutput=False)
    wp = nc.declare_dram_parameter("wp", [128, D], F32, isOutput=False)
    bias = nc.declare_dram_parameter("bias", [128, 3], F32, isOutput=False)
    cst = nc.declare_dram_parameter("cst", [1, 64], F32, isOutput=False)
    out = nc.declare_dram_parameter("out", [T, D], F32, isOutput=True)

    with tile.TileContext(nc) as tc:
        with (
            tc.tile_pool(name="persist", bufs=1) as persist,
            tc.tile_pool(name="wpool", bufs=1) as wpool,
            tc.tile_pool(name="xtp", bufs=3) as xtp,
            tc.tile_pool(name="expp", bufs=4) as expp,
            tc.tile_pool(name="smp", bufs=4) as smp,
            tc.tile_pool(name="osbp", bufs=4) as osbp,
            tc.tile_pool(name="drp", bufs=4, space="DRAM") as drp,
            tc.tile_pool(name="psum", bufs=2, space="PSUM") as psum,
        ):
            # weights first in DMA queue order: one strided DMA each packs
            # all 8 contraction chunks [128, 8, n] so the DGE issues fewer,
            # larger transfers (dst partition p, chunk i <- DRAM row 128i+p).
            # weights first in DMA queue order. Chunk 0 of wqk is split out
            # so the very first q-matmul only waits on a 64 KB transfer.
            wqk_all = wpool.tile([128, 8, 256], BF16, tag="wqk_all")
            nc.sync.dma_start(
                out=wqk_all[:, 0:1, :],
                in_=bass.AP(
                    tensor=wqk[0:1, 0:1].tensor, offset=0,
                    ap=[[256, 128], [128 * 256, 1], [1, 256]],
                ),
            )
            wqk_rest = lambda: nc.sync.dma_start(
                out=wqk_all[:, 1:8, :],
                in_=bass.AP(
                    tensor=wqk[0:1, 0:1].tensor, offset=128 * 256,
                    ap=[[256, 128], [128 * 256, 7], [1, 256]],
                ),
            )
            wv_all = wpool.tile([128, 8, 128], BF16, tag="wv_all")
            wv_load = lambda: nc.sync.dma_start(
                out=wv_all,
                in_=bass.AP(
                    tensor=wv[0:1, 0:1].tensor, offset=0,
                    ap=[[128, 128], [128 * 128, 8], [1, 128]],
                ),
            )
            w_qk = [wqk_all[:, i, :] for i in range(8)]
            w_v = [wv_all[:, i, :] for i in range(8)]

            bias_sb = persist.tile([128, 3], F32, tag="bias_sb")
            ones64 = persist.tile([1, 64], F32R, tag="ones64")

            qT = persist.tile([128, T], F32R, tag="qT")
            kT = persist.tile([128, T], F32R, tag="kT")
            # v3[:, g, :] = [vA | 1 | vB | 1] for global 128-token block g:
            # head h uses cols 65h:65h+65 ([v|1]); AV output row 64 = sumexp.
            v3 = persist.tile([128, T // 128, 130], F32R, tag="v3")
            aot = [
                persist.tile([128, S], F32R, tag=f"aot{b}", name=f"aot{b}")
                for b in range(B)
            ]
            wp_sb = persist.tile([128, D], F32R, tag="wp_sb")

            strip_x = {}

            strip_tiles = {}

            def load_strip_part(t, lo, hi):
                c0 = (t % 8) * 512
                if t not in strip_tiles:
                    xall = xtp.tile(
                        [128, 8, 512], BF16, tag="xall", name="xall"
                    )
                    strip_tiles[t] = xall
                    strip_x[t] = [xall[:, i, :] for i in range(8)]
                nc.sync.dma_start(
                    out=strip_tiles[t][:, lo:hi, :],
                    in_=bass.AP(
                        tensor=xT[0:1, 0:1].tensor,
                        offset=lo * 128 * T + c0,
                        ap=[[T, 128], [128 * T, hi - lo], [1, 512]],
                    ),
                )

            def load_strip(t):
                load_strip_part(t, 0, 8)

            def emit_small_consts():
                for col in (64, 129):
                    csrc = bass.AP(
                        tensor=cst[0:1, 0:32].tensor,
                        offset=cst[0:1, 0:32].offset,
                        ap=[[0, 128], [1, T // 128], [0, 1]],
                    )
                    nc.sync.dma_start(
                        out=v3[:, :, col : col + 1], in_=csrc.bitcast(F32R)
                    )
                nc.sync.dma_start(out=bias_sb, in_=bias[:, :])
                nc.sync.dma_start(out=ones64, in_=cst[0:1, :].bitcast(F32R))

            def strip_closures(t):
                """48 single-PE-op closures for one 512-token strip: q chain,
                k chain, then 4 direct-transposed-V chains (tok-block each).
                Chains stay contiguous in the filler stream so the shared
                "mm" psum tag has at most one open accumulation chain."""
                c0 = (t % 8) * 512
                xs = strip_x[t]
                ops = []

                def qk_op(i, m, dst):
                    def go():
                        if i == 0:
                            qk_op.p = psum.tile(
                                [128, 512], F32, tag="mm", name="pqk"
                            )
                        nc.tensor.matmul(
                            qk_op.p,
                            lhsT=w_qk[i][:, m * 128 : (m + 1) * 128],
                            rhs=xs[i],
                            start=(i == 0),
                            stop=(i == 7),
                        )
                        if i == 7:
                            if zero_bias:
                                with nc.allow_low_precision(reason="f32r qk"):
                                    nc.vector.tensor_copy(
                                        out=dst[:, c0 : c0 + 512], in_=qk_op.p
                                    )
                            else:
                                with nc.allow_low_precision(reason="f32r qk"):
                                    nc.vector.tensor_scalar(
                                        out=dst[:, c0 : c0 + 512],
                                        in0=qk_op.p,
                                        scalar1=bias_sb[:, m : m + 1],
                                        scalar2=None,
                                        op0=ADD,
                                    )
                    return go

                for m, dst in ((0, qT), (1, kT)):
                    for i in range(8):
                        ops.append(qk_op(i, m, dst))

                def v_op(i, s4):
                    g = (t % 8) * 4 + s4

                    def go():
                        if i == 0:
                            v_op.p = psum.tile(
                                [128, 128], F32, tag="mm", name="pv"
                            )
                        nc.tensor.matmul(
                            v_op.p,
                            lhsT=xs[i][:, s4 * 128 : (s4 + 1) * 128],
                            rhs=w_v[i],
                            start=(i == 0),
                            stop=(i == 7),
                        )
                        if i == 7:
                            with nc.allow_low_precision(reason="f32r v3"):
                                nc.vector.tensor_copy(
                                    out=v3[:, g, 0:64], in_=v_op.p[:, 0:64]
                                )
                                nc.vector.tensor_copy(
                                    out=v3[:, g, 65:129], in_=v_op.p[:, 64:128]
                                )
                    return go

                for s4 in range(4):
                    for i in range(8):
                        ops.append(v_op(i, s4))
                return ops

            def proj_closures(b, qc, split_store=False):
                """8 single-PE-op closures: partial projection of one
                512-token strip of aot[b]. Each op allocs+drains its own
                "mm" psum tile (no open chain across pulls). split_store
                stores each half as soon as it drains (tail latency)."""
                ops = []

                def p_op(t4, n2):
                    col0 = qc * 512 + t4 * 128
                    row = b * S + col0

                    def go():
                        if n2 == 0:
                            p_op.osb = osbp.tile(
                                [128, D], F32, tag="osb", name="osb"
                            )
                        pp = psum.tile([128, 512], F32, tag="mm", name="pp")
                        nc.tensor.matmul(
                            pp,
                            lhsT=aot[b][:, col0 : col0 + 128],
                            rhs=wp_sb[:, n2 * 512 : (n2 + 1) * 512],
                            start=True,
                            stop=True,
                        )
                        nc.vector.tensor_copy(
                            out=p_op.osb[:, n2 * 512 : (n2 + 1) * 512], in_=pp
                        )
                        if split_store:
                            nc.sync.dma_start(
                                out=out[row : row + 128,
                                        n2 * 512 : (n2 + 1) * 512],
                                in_=p_op.osb[:, n2 * 512 : (n2 + 1) * 512],
                            )
                        elif n2 == 1:
                            nc.sync.dma_start(
                                out=out[row : row + 128, :], in_=p_op.osb
                            )
                    return go

                for t4 in range(4):
                    for n2 in range(2):
                        ops.append(p_op(t4, n2))
                return ops

            def emit_attn(b, qc, fillers, hold=0, pe_norm=False):
                """One attention unit: 16 key-chunks, 1-chunk software
                pipeline with PE fillers pulled between scores and AV.
                `hold` fillers are kept back until after the last AV so the
                PE queue stays fed while the normalization chain runs."""
                held = fillers[len(fillers) - hold:] if hold else []
                fillers = fillers[: len(fillers) - hold]
                q0 = b * S + qc * 512
                avs = [
                    psum.tile([65, 512], F32, tag=f"av{h}", bufs=1,
                              name=f"av{h}")
                    for h in range(HPC)
                ]
                exs = [None] * 16

                def sc_exp(kb):
                    # Scores for both heads into one 2-bank tile + one
                    # merged exp. The "s" alloc (2 bufs) makes sc(k) wait on
                    # exp(k-2); emitting av(k-2) AFTER sc(k) then keeps the
                    # ACT-bound phases paced by exp alone — av(k-2) is
                    # already ready when its queue turn comes.
                    k0 = b * S + kb * 128
                    ps = psum.tile([128, 1024], F32, tag="s", name="pscore")
                    for h in range(HPC):
                        hr = slice(h * 64, (h + 1) * 64)
                        nc.tensor.matmul(
                            ps[:, h * 512 : (h + 1) * 512],
                            lhsT=kT[hr, k0 : k0 + 128],
                            rhs=qT[hr, q0 : q0 + 512],
                            start=True,
                            stop=True,
                        )
                    if skip_exp:
                        # attribution probe: half-width exp, shared by both
                        # heads (wrong values, ACT work halved)
                        ex = expp.tile([128, 512], F32R, tag="exp", name="ex")
                        nc.scalar.activation(out=ex, in_=ps[:, 0:512],
                                             func=Exp)
                    else:
                        ex = expp.tile([128, 1024], F32R, tag="exp", name="ex")
                        nc.scalar.activation(out=ex, in_=ps, func=Exp)
                    exs[kb] = ex

                def av_pair(kb):
                    g = b * 16 + kb
                    for h in range(HPC):
                        rhs = (exs[kb] if skip_exp
                               else exs[kb][:, h * 512 : (h + 1) * 512])
                        nc.tensor.matmul(
                            avs[h],
                            lhsT=v3[:, g, 65 * h : 65 * h + 65],
                            rhs=rhs,
                            start=(kb == 0),
                            stop=(kb == 15),
                        )

                def pull(n):
                    for _ in range(min(n, len(fillers))):
                        fillers.pop(0)()

                for kb in range(16):
                    sc_exp(kb)
                    base = 4 if kb < 2 else 2
                    pace = (len(fillers) + 15 - kb) // (16 - kb)  # ceil
                    npull = max(base, pace)
                    pull(npull - npull // 2)
                    if kb >= 2:
                        av_pair(kb - 2)
                    pull(npull // 2)
                av_pair(14)
                av_pair(15)
                for op in held:
                    op()
                while fillers:
                    fillers.pop(0)()
                # normalize: rc = 1/sumexp broadcast across 64 partitions,
                # then scale the AV rows into aot. The AV psum tile is
                # drained to SBUF right after its recip so the single-
                # buffered accumulator bank frees after ~1.3 us instead of
                # waiting on the whole broadcast+mul chain — the next AU's
                # first AV matmuls then start without stalling.
                bcss, avsbs = [], []
                for h in range(HPC):
                    rc = smp.tile([1, 512], F32R, tag="rc", name="rc")
                    with nc.allow_low_precision(reason="softmax recip"):
                        nc.vector.reciprocal(out=rc, in_=avs[h][64:65, :])
                    avsb = smp.tile([64, 512], F32, tag="avsb", name="avsb")
                    if pe_norm:
                        # tail AU: ACT is done with exps — drain there so
                        # the DVE queue only holds recips and muls
                        nc.scalar.copy(out=avsb, in_=avs[h][0:64, :])
                    else:
                        nc.vector.tensor_copy(out=avsb, in_=avs[h][0:64, :])
                    avsbs.append(avsb)
                    if pe_norm:
                        # tail-only: K=1 PE matmul broadcast — PE is idle
                        # waiting on this chain anyway and the latency is
                        # ~1/3 of the DMA bounce
                        bcs = psum.tile([64, 512], F32, tag="mm", name="rcb")
                        nc.tensor.matmul(
                            bcs, lhsT=ones64, rhs=rc, start=True, stop=True
                        )
                        bcss.append(bcs)
                        continue
                    bcs = smp.tile([64, 512], F32R, tag="bcs", name="bcs")
                    if pbcast == "gpsimd":
                        nc.gpsimd.partition_broadcast(bcs, rc, channels=64)
                    else:  # "bounce"
                        scr = drp.tile([1, 512], F32R, tag="scr", name="scr")
                        nc.sync.dma_start(out=scr, in_=rc)
                        bsrc = bass.AP(
                            tensor=scr.tensor,
                            offset=scr.offset,
                            ap=[[0, 64], [1, 512]],
                        )
                        nc.sync.dma_start(out=bcs, in_=bsrc)
                    bcss.append(bcs)
                for h in range(HPC):
                    hr = slice(h * 64, (h + 1) * 64)
                    with nc.allow_low_precision(reason="f32r attn out"):
                        nc.vector.tensor_mul(
                            out=aot[b][hr, qc * 512 : (qc + 1) * 512],
                            in0=avsbs[h],
                            in1=bcss[h],
                        )
                    if not zero_bias:
                        with nc.allow_low_precision(reason="v bias"):
                            nc.vector.tensor_scalar(
                                out=aot[b][hr, qc * 512 : (qc + 1) * 512],
                                in0=aot[b][hr, qc * 512 : (qc + 1) * 512],
                                scalar1=bias_sb[hr, 2:3],
                                scalar2=None,
                                op0=ADD,
                            )

            for r in range(reps):
                # phase 1: batch-0 QKV strips (PE-dense, ACT idle).
                if r == 0:
                    # startup order: wqk chunk 0 is already queued; stage
                    # the first x half before the bulk weight loads so the
                    # q-chain can start ~4 us earlier.
                    load_strip_part(0, 0, 4)
                    wqk_rest()
                    wv_load()
                    load_strip_part(0, 4, 8)
                    load_strip(1)
                    emit_small_consts()
                else:
                    load_strip(r * 8 + 0)
                    load_strip(r * 8 + 1)
                for t in range(4):
                    if t + 2 <= 7:
                        load_strip(r * 8 + t + 2)
                    for op in strip_closures(r * 8 + t):
                        op()
                if r == 0:
                    nc.sync.dma_start(out=wp_sb, in_=wp[:, :].bitcast(F32R))
                # phase 2: batch-0 attention with batch-1 QKV as fillers
                # (PE-bound at ~1172 ns/chunk vs ACT's 1038).
                for qc in range(4):
                    if qc + 6 <= 7:
                        load_strip(r * 8 + qc + 6)
                    emit_attn(0, qc, strip_closures(r * 8 + 4 + qc))
                # phase 3: batch-1 attention; ALL 56 remaining projection
                # ops spread 14 per AU so per-chunk PE work (~1038 ns)
                # exactly matches the exp rate — the ACT-paced phase runs
                # with no dry PE queue. Splits stay on even (pair) bounds.
                p00, p01, p02, p03 = (proj_closures(0, i) for i in range(4))
                p10, p11, p12 = (proj_closures(1, i) for i in range(3))
                fl = [
                    p00 + p01[:6],              # AU(1,0)
                    p01[6:] + p02 + p10[:4],    # AU(1,1)
                    p10[4:] + p03 + p11[:2],    # AU(1,2)
                    p11[2:] + p12,              # AU(1,3)
                ]
                for qc in range(4):
                    emit_attn(1, qc, fl[qc], hold=8 if qc == 3 else 0,
                              pe_norm=(qc == 3))
                for op in proj_closures(1, 3, split_store=True):
                    op()

    nc.compile()
    _NC_CACHE[key] = nc
    return nc


def prep_inputs(x, Wqkv, bqkv, Wproj):
    """Host-side sharding: returns the per-core input maps."""
    import ml_dtypes

    BF = ml_dtypes.bfloat16
    x = np.asarray(x, dtype=np.float32)
    Wqkv = np.asarray(Wqkv, dtype=np.float32)
    bqkv = np.asarray(bqkv, dtype=np.float32)
    Wproj = np.asarray(Wproj, dtype=np.float32)

    xT = np.ascontiguousarray(x.reshape(T, D).T).astype(BF)
    qk_scale = np.float32(HD ** -0.25)
    cst = np.ones((1, 64), dtype=np.float32)

    in_maps = []
    for c in range(N_CORES):
        f0 = c * HPC * HD  # first local feature column
        wqk_c = np.concatenate(
            [Wqkv[:, f0 : f0 + 128], Wqkv[:, D + f0 : D + f0 + 128]], axis=1
        ) * qk_scale
        wv_c = Wqkv[:, 2 * D + f0 : 2 * D + f0 + 128]
        wp_c = Wproj[f0 : f0 + 128, :]
        bias_c = np.stack(
            [
                bqkv[f0 : f0 + 128] * qk_scale,
                bqkv[D + f0 : D + f0 + 128] * qk_scale,
                bqkv[2 * D + f0 : 2 * D + f0 + 128],
            ],
            axis=1,
        )
        in_maps.append(
            {
                "xT": xT,
                "wqk": np.ascontiguousarray(wqk_c).astype(BF),
                "wv": np.ascontiguousarray(wv_c).astype(BF),
                "wp": np.ascontiguousarray(wp_c),
                "bias": np.ascontiguousarray(bias_c),
                "cst": cst,
            }
        )
    return in_maps


def kernel(x, Wqkv, bqkv, Wproj, bproj):
    from concourse.bass_utils import run_bass_kernel_spmd

    zb = not np.asarray(bqkv).any()
    nc = build_nc(zero_bias=zb)
    in_maps = prep_inputs(x, Wqkv, bqkv, Wproj)
    res = run_bass_kernel_spmd(nc, in_maps, core_ids=list(range(N_CORES)))
    total = res.results[0]["out"].astype(np.float32).copy()
    for c in range(1, N_CORES):
        total += res.results[c]["out"]
    total += np.asarray(bproj, dtype=np.float32)[None, :]
    return total.reshape(B, S, D)


if __name__ == "__main__":
    rng = np.random.default_rng(0)
    x = rng.standard_normal((B, S, D)).astype(np.float32)
    Wqkv = (rng.standard_normal((D, 3 * D)) * D**-0.5).astype(np.float32)
    bqkv = np.zeros(3 * D, np.float32)
    Wproj = (rng.standard_normal((D, D)) * D**-0.5).astype(np.float32)
    bproj = np.zeros(D, np.float32)
    got = kernel(x, Wqkv, bqkv, Wproj, bproj)
    print("ran ok", got.shape, got.dtype)



# revision 3
# speedup vs baseline: 1.1462x; 1.1462x over previous
"""Multi-head attention (B=2, S=2048, D=1024, H=16) on 8 Trainium2 cores.

Sharding: tensor-parallel over heads — 2 heads per core. Each core computes
QKV for its 384 features (contraction over D with the full x), per-head
attention (scores -> exp -> AV, softmax without max-subtraction since scores
are O(1)), and a partial output projection against its 128 rows of Wproj.
The host sums the 8 partial bf16 projections and adds bproj.

All matmul operands are bf16 (qT/kT/v3/aot/wp and the exp output); PSUM
accumulation stays f32. Scores are zero-padded to K=128 (kT stored twice,
each head's 64 features padded with a zeroed half) — K=64 matmuls measured
~2x slower per streamed row on HW. Partial outputs are stored bf16 (halves
the 16MB/core store traffic); the host-side sum upcasts to f32.

Measured on HW (A/B NEFF differencing): the per-chunk exp [128,1024]
(PSUM->SBUF) costs ~1.38us — 33% above the cost model — making ACT the
pacing engine for the attention phases (~177us of exp per core). The
schedule therefore: phase 1 computes only strips 0-1 of batch-0 QKV before
attention starts (strips 2-3 k/v ride as AU(0,0) fillers, q2/q3 in
AU(0,1)), batch-1 QKV fills batch-0's attention, and the projection fills
batch-1's. The softmax reciprocal broadcast stays on the DMA bounce path
(SBUF->DRAM->SBUF) except the last AU, which uses a K=1 PE matmul.
"""

import numpy as np

B, S, D, H = 2, 2048, 1024, 16
HD = D // H          # 64
T = B * S            # 4096 tokens
N_CORES = 8
HPC = H // N_CORES   # 2 heads per core

KD = [(i * 128, 128) for i in range(8)]  # contraction chunks over D

_NC_CACHE = {}


def build_nc(reps: int = 1, zero_bias: bool = True, pbcast: str = "bounce",
             skip_exp: bool = False, store: str = "bf16", k64: str = "pad",
             early: bool = True, vt: bool = False, dummy: int = 0,
             drains: str = "dve", pdrain: str = "dve",
             probe: str = "", vbatch: bool = False,
             esplit: bool = False, exf32: bool = False):
    """pbcast: how the per-query softmax reciprocal [1,512] is broadcast
    across 64 partitions — "gpsimd" (partition_broadcast) or "bounce"
    (SBUF->DRAM->SBUF DMA pair; partition-stride-0 SBUF sources are not
    allowed so a direct SBUF->SBUF broadcast DMA is impossible).
    skip_exp: timing-attribution probe — drop the exp (AV reads raw
    scores; results are wrong).
    store: "f32" | "bf16" (halve partial-store DMA) | "none" (probe:
    skip output stores entirely; results are wrong)."""
    key = (reps, zero_bias, pbcast, skip_exp, store, k64, early, vt, dummy, drains, pdrain, probe, vbatch, esplit, exf32)
    if key in _NC_CACHE:
        return _NC_CACHE[key]

    from concourse import bacc
    import concourse.bass as bass
    import concourse.mybir as mybir
    import concourse.tile as tile

    F32 = mybir.dt.float32
    F32R = mybir.dt.float32r
    BF16 = mybir.dt.bfloat16
    Exp = mybir.ActivationFunctionType.Exp
    ADD = mybir.AluOpType.add

    nc = bacc.Bacc()
    xT = nc.declare_dram_parameter("xT", [D, T], BF16, isOutput=False)
    wqk = nc.declare_dram_parameter("wqk", [D, 256], BF16, isOutput=False)
    wv = nc.declare_dram_parameter("wv", [D, 128], BF16, isOutput=False)
    wp = nc.declare_dram_parameter("wp", [128, D], BF16, isOutput=False)
    bias = nc.declare_dram_parameter("bias", [128, 3], F32, isOutput=False)
    cst = nc.declare_dram_parameter("cst", [1, 64], F32, isOutput=False)
    OUT_DT = BF16 if store == "bf16" else F32
    out = nc.declare_dram_parameter("out", [T, D], OUT_DT, isOutput=True)

    with tile.TileContext(nc) as tc:
        with (
            tc.tile_pool(name="persist", bufs=1) as persist,
            tc.tile_pool(name="wpool", bufs=1) as wpool,
            tc.tile_pool(name="xtp", bufs=3) as xtp,
            tc.tile_pool(name="expp", bufs=4) as expp,
            tc.tile_pool(name="vsbp", bufs=2) as vsbp,
            tc.tile_pool(name="smp", bufs=4) as smp,
            tc.tile_pool(name="osbp", bufs=4) as osbp,
            tc.tile_pool(name="drp", bufs=4, space="DRAM") as drp,
            tc.tile_pool(name="psum", bufs=2, space="PSUM") as psum,
        ):
            # weights first in DMA queue order: one strided DMA each packs
            # all 8 contraction chunks [128, 8, n] so the DGE issues fewer,
            # larger transfers (dst partition p, chunk i <- DRAM row 128i+p).
            # weights first in DMA queue order. Chunk 0 of wqk is split out
            # so the very first q-matmul only waits on a 64 KB transfer.
            wqk_all = wpool.tile([128, 8, 256], BF16, tag="wqk_all")
            nc.sync.dma_start(
                out=wqk_all[:, 0:1, :],
                in_=bass.AP(
                    tensor=wqk[0:1, 0:1].tensor, offset=0,
                    ap=[[256, 128], [128 * 256, 1], [1, 256]],
                ),
            )
            wqk_rest = lambda: nc.sync.dma_start(
                out=wqk_all[:, 1:8, :],
                in_=bass.AP(
                    tensor=wqk[0:1, 0:1].tensor, offset=128 * 256,
                    ap=[[256, 128], [128 * 256, 7], [1, 256]],
                ),
            )
            wv_all = wpool.tile([128, 8, 128], BF16, tag="wv_all")
            wv_load = lambda: nc.sync.dma_start(
                out=wv_all,
                in_=bass.AP(
                    tensor=wv[0:1, 0:1].tensor, offset=0,
                    ap=[[128, 128], [128 * 128, 8], [1, 128]],
                ),
            )
            w_qk = [wqk_all[:, i, :] for i in range(8)]
            w_v = [wv_all[:, i, :] for i in range(8)]

            bias_sb = persist.tile([128, 3], F32, tag="bias_sb")
            ones64 = persist.tile([1, 64], F32R, tag="ones64")

            qT = persist.tile([128, T], BF16, tag="qT")
            if k64 == "pad":
                kT = persist.tile([128, 2, T], BF16, tag="kTp")
                nc.vector.memset(kT[64:128, 0, :], 0.0)
                nc.vector.memset(kT[0:64, 1, :], 0.0)
            else:
                kT = persist.tile([128, T], BF16, tag="kT")
            # v3[:, g, :] = [vA | 1 | vB | 1] for global 128-token block g:
            # head h uses cols 65h:65h+65 ([v|1]); AV output row 64 = sumexp.
            V3DT = F32R if exf32 else BF16
            v3 = persist.tile([128, T // 128, 130], V3DT, tag="v3")
            aot = [
                persist.tile([128, S], BF16, tag=f"aot{b}", name=f"aot{b}")
                for b in range(B)
            ]
            wp_sb = persist.tile([128, D], BF16, tag="wp_sb")

            strip_x = {}

            strip_tiles = {}

            def load_strip_part(t, lo, hi):
                c0 = (t % 8) * 512
                if t not in strip_tiles:
                    xall = xtp.tile(
                        [128, 8, 512], BF16, tag="xall", name="xall"
                    )
                    strip_tiles[t] = xall
                    strip_x[t] = [xall[:, i, :] for i in range(8)]
                nc.sync.dma_start(
                    out=strip_tiles[t][:, lo:hi, :],
                    in_=bass.AP(
                        tensor=xT[0:1, 0:1].tensor,
                        offset=lo * 128 * T + c0,
                        ap=[[T, 128], [128 * T, hi - lo], [1, 512]],
                    ),
                )

            def load_strip(t):
                load_strip_part(t, 0, 8)

            def emit_small_consts():
                ocols = (0, 129) if vt else (64, 129)
                for col in ocols:
                    tgt = v3[:, :, col : col + 1]
                    if exf32:
                        tgt = tgt.bitcast(F32)
                    nc.vector.memset(tgt, 1.0)
                nc.sync.dma_start(out=bias_sb, in_=bias[:, :])
                nc.sync.dma_start(out=ones64, in_=cst[0:1, :].bitcast(F32R))

            def strip_parts(t):
                """(q_ops, k_ops, v_ops) closures for one 512-token strip.
                Chains stay contiguous in the filler stream so the shared
                "mm" psum tag has at most one open accumulation chain."""
                c0 = (t % 8) * 512
                xs = strip_x[t]
                ops = []

                def qk_op(i, m, dst):
                    def go():
                        if i == 0:
                            qk_op.p = psum.tile(
                                [128, 512], F32, tag="mm", name="pqk"
                            )
                        nc.tensor.matmul(
                            qk_op.p,
                            lhsT=w_qk[i][:, m * 128 : (m + 1) * 128],
                            rhs=xs[i],
                            start=(i == 0),
                            stop=(i == 7),
                        )
                        if i == 7:
                            cp = (nc.scalar.copy if drains == "act"
                                  else nc.vector.tensor_copy)
                            with nc.allow_low_precision(reason="bf16 qk"):
                                if not zero_bias:
                                    nc.vector.tensor_scalar(
                                        out=qk_op.p,
                                        in0=qk_op.p,
                                        scalar1=bias_sb[:, m : m + 1],
                                        scalar2=None,
                                        op0=ADD,
                                    )
                                if m == 1 and k64 == "pad":
                                    cp(
                                        out=dst[0:64, 0, c0 : c0 + 512],
                                        in_=qk_op.p[0:64, :],
                                    )
                                    cp(
                                        out=dst[64:128, 1, c0 : c0 + 512],
                                        in_=qk_op.p[64:128, :],
                                    )
                                else:
                                    cp(
                                        out=dst[:, c0 : c0 + 512], in_=qk_op.p
                                    )
                    return go

                for m, dst in ((0, qT), (1, kT)):
                    for i in range(8):
                        ops.append(qk_op(i, m, dst))

                def v_op(i, s4):
                    g = (t % 8) * 4 + s4

                    def go():
                        if i == 0:
                            v_op.p = psum.tile(
                                [128, 128], F32, tag="mm", name="pv"
                            )
                        nc.tensor.matmul(
                            v_op.p,
                            lhsT=xs[i][:, s4 * 128 : (s4 + 1) * 128],
                            rhs=w_v[i],
                            start=(i == 0),
                            stop=(i == 7),
                        )
                        if i == 7:
                            cp = (nc.scalar.copy if drains == "act"
                                  else nc.vector.tensor_copy)
                            with nc.allow_low_precision(reason="bf16 v3"):
                                cp(
                                    out=v3[:, g, 0:64], in_=v_op.p[:, 0:64]
                                )
                                cp(
                                    out=v3[:, g, 65:129], in_=v_op.p[:, 64:128]
                                )
                    return go

                qk = ops

                vops = []
                if vbatch:
                    # all 4 token-blocks' V chains share one psum bank
                    # (disjoint 128-col quarters, 4 open accumulation
                    # chains), drained by 2 strided copies instead of 8.
                    def vb_op(i):
                        def go():
                            if i == 0:
                                vb_op.p = psum.tile(
                                    [128, 4, 128], F32, tag="mm", name="pvb"
                                )
                            for s4 in range(4):
                                nc.tensor.matmul(
                                    vb_op.p[:, s4, :],
                                    lhsT=xs[i][:, s4 * 128 : (s4 + 1) * 128],
                                    rhs=w_v[i],
                                    start=(i == 0),
                                    stop=(i == 7),
                                    skip_group_check=True,
                                )
                            if i == 7:
                                g0 = (t % 8) * 4
                                cp = (nc.scalar.copy if drains == "act"
                                      else nc.vector.tensor_copy)
                                with nc.allow_low_precision(reason="bf16 v3"):
                                    cp(out=v3[:, g0 : g0 + 4, 0:64],
                                       in_=vb_op.p[:, :, 0:64])
                                    cp(out=v3[:, g0 : g0 + 4, 65:129],
                                       in_=vb_op.p[:, :, 64:128])
                        return go

                    for i in range(8):
                        vops.append(vb_op(i))
                elif vt:
                    # vT chains (N=512) + DMA xbar transpose into v3
                    def vt_op(i):
                        def go():
                            if i == 0:
                                vt_op.p = psum.tile(
                                    [128, 512], F32, tag="mm", name="pvt"
                                )
                            nc.tensor.matmul(
                                vt_op.p,
                                lhsT=w_v[i],
                                rhs=xs[i],
                                start=(i == 0),
                                stop=(i == 7),
                            )
                            if i == 7:
                                vsb = vsbp.tile(
                                    [128, 512], BF16, tag="vsb", name="vsb"
                                )
                                with nc.allow_low_precision(reason="bf16 v"):
                                    nc.vector.tensor_copy(out=vsb, in_=vt_op.p)
                                g0 = (t % 8) * 4
                                for gr in range(4):
                                    nc.sync.dma_start_transpose(
                                        out=v3[:, g0 + gr, 1:129],
                                        in_=vsb[:, 128 * gr : 128 * gr + 128],
                                    )
                        return go

                    for i in range(8):
                        vops.append(vt_op(i))
                else:
                    for s4 in range(4):
                        for i in range(8):
                            vops.append(v_op(i, s4))
                return qk[0:8], qk[8:16], vops

            def strip_closures(t):
                q, k, v = strip_parts(t)
                return q + k + v

            def proj_closures(b, qc, split_store=False):
                """8 single-PE-op closures: partial projection of one
                512-token strip of aot[b]. Each op allocs+drains its own
                "mm" psum tile (no open chain across pulls). split_store
                stores each half as soon as it drains (tail latency)."""
                ops = []

                def p_op(t4, n2):
                    col0 = qc * 512 + t4 * 128
                    row = b * S + col0

                    def go():
                        if n2 == 0:
                            p_op.osb = osbp.tile(
                                [128, D], OUT_DT, tag="osb", name="osb"
                            )
                        pp = psum.tile([128, 512], F32, tag="mm", name="pp")
                        nc.tensor.matmul(
                            pp,
                            lhsT=aot[b][:, col0 : col0 + 128],
                            rhs=wp_sb[:, n2 * 512 : (n2 + 1) * 512],
                            start=True,
                            stop=True,
                        )
                        pcp = {"dve": nc.vector.tensor_copy,
                               "act": nc.scalar.copy,
                               "pool": nc.gpsimd.tensor_copy}[pdrain]
                        with nc.allow_low_precision(reason="partial store"):
                            pcp(
                                out=p_op.osb[:, n2 * 512 : (n2 + 1) * 512],
                                in_=pp,
                            )
                        if store == "none":
                            return
                        if split_store:
                            nc.sync.dma_start(
                                out=out[row : row + 128,
                                        n2 * 512 : (n2 + 1) * 512],
                                in_=p_op.osb[:, n2 * 512 : (n2 + 1) * 512],
                            )
                        elif n2 == 1:
                            nc.sync.dma_start(
                                out=out[row : row + 128, :], in_=p_op.osb
                            )
                    return go

                for t4 in range(4):
                    for n2 in range(2):
                        ops.append(p_op(t4, n2))
                return ops

            def emit_attn(b, qc, fillers, hold=0, pe_norm=False,
                          front=0, av_last=False):
                """One attention unit: 16 key-chunks, 1-chunk software
                pipeline with PE fillers pulled between scores and AV.
                `hold` fillers are kept back until after the last AV so the
                PE queue stays fed while the normalization chain runs."""
                held = fillers[len(fillers) - hold:] if hold else []
                fillers = fillers[: len(fillers) - hold]
                q0 = b * S + qc * 512
                avs = [
                    psum.tile([65, 512], F32, tag=f"av{h}", bufs=1,
                              name=f"av{h}")
                    for h in range(HPC)
                ]
                exs = [None] * 16

                def sc_exp(kb):
                    # Scores for both heads into one 2-bank tile + one
                    # merged exp. The "s" alloc (2 bufs) makes sc(k) wait on
                    # exp(k-2); emitting av(k-2) AFTER sc(k) then keeps the
                    # ACT-bound phases paced by exp alone — av(k-2) is
                    # already ready when its queue turn comes.
                    k0 = b * S + kb * 128
                    ps = psum.tile([128, 1024], F32, tag="s", name="pscore")
                    for h in range(HPC):
                        hr = slice(h * 64, (h + 1) * 64)
                        if k64 == "pad":
                            nc.tensor.matmul(
                                ps[:, h * 512 : (h + 1) * 512],
                                lhsT=kT[:, h, k0 : k0 + 128],
                                rhs=qT[:, q0 : q0 + 512],
                                start=True,
                                stop=True,
                            )
                        else:
                            nc.tensor.matmul(
                                ps[:, h * 512 : (h + 1) * 512],
                                lhsT=kT[hr, k0 : k0 + 128],
                                rhs=qT[hr, q0 : q0 + 512],
                                start=True,
                                stop=True,
                            )
                    if skip_exp:
                        # attribution probe: half-width exp, shared by both
                        # heads (wrong values, ACT work halved)
                        ex = expp.tile([128, 512], BF16, tag="exp", name="ex")
                        nc.scalar.activation(out=ex, in_=ps[:, 0:512],
                                             func=Exp)
                    else:
                        EDT = F32R if exf32 else BF16
                        ex = expp.tile([128, 1024], EDT, tag="exp", name="ex")
                        if esplit:
                            # one exp per psum bank: a 2-bank-spanning AP
                            # may read slower than two single-bank reads
                            nc.scalar.activation(
                                out=ex[:, 0:512], in_=ps[:, 0:512], func=Exp
                            )
                            nc.scalar.activation(
                                out=ex[:, 512:1024], in_=ps[:, 512:1024],
                                func=Exp,
                            )
                        else:
                            nc.scalar.activation(out=ex, in_=ps, func=Exp)
                    exs[kb] = ex

                def av_pair(kb):
                    g = b * 16 + kb
                    for h in range(HPC):
                        rhs = (exs[kb] if skip_exp
                               else exs[kb][:, h * 512 : (h + 1) * 512])
                        nc.tensor.matmul(
                            avs[h],
                            lhsT=v3[:, g, 65 * h : 65 * h + 65],
                            rhs=rhs,
                            start=(kb == 0),
                            stop=(kb == 15),
                        )

                def se_row(h):
                    # vt layout: h0 stationary is [1|v] (sumexp row 0);
                    # h1 is [v|1] (row 64). v_op layout: row 64 for both.
                    return 0 if (vt and h == 0) else 64

                def v_rows(h):
                    return slice(1, 65) if (vt and h == 0) else slice(0, 64)

                def pull(n):
                    for _ in range(min(n, len(fillers))):
                        fillers.pop(0)()

                def dummy_op():
                    # p-state keep-alive: standalone bf16 ldweights — a PE
                    # instruction with no psum output and no waits (the next
                    # real matmul self-loads its own weights, so clobbering
                    # the weight register is harmless). Keeps the PE busy so
                    # the clock stays at max through ACT-paced stretches.
                    nc.tensor.ldweights(wp_sb[:, 0:128])

                for kb in range(16):
                    sc_exp(kb)
                    base = 4 if kb < 2 else 2
                    if front and kb < 6:
                        base = front
                    pace = (len(fillers) + 15 - kb) // (16 - kb)  # ceil
                    npull = max(base, pace)
                    if av_last:
                        pull(npull)
                        if kb >= 2:
                            av_pair(kb - 2)
                    else:
                        pull(npull - npull // 2)
                        if kb >= 2:
                            av_pair(kb - 2)
                        pull(npull // 2)
                    for _ in range(dummy):
                        dummy_op()
                av_pair(14)
                av_pair(15)
                for op in held:
                    op()
                while fillers:
                    fillers.pop(0)()
                # normalize: rc = 1/sumexp broadcast across 64 partitions,
                # then scale the AV rows into aot. The AV psum tile is
                # drained to SBUF right after its recip so the single-
                # buffered accumulator bank frees after ~1.3 us instead of
                # waiting on the whole broadcast+mul chain — the next AU's
                # first AV matmuls then start without stalling.
                bcss, avsbs = [], []
                for h in range(HPC):
                    sr = se_row(h)
                    vr = v_rows(h)
                    rc = smp.tile([1, 512], F32R, tag="rc", name="rc")
                    with nc.allow_low_precision(reason="softmax recip"):
                        nc.vector.reciprocal(out=rc, in_=avs[h][sr : sr + 1, :])
                    avsb = smp.tile([64, 512], F32, tag="avsb", name="avsb")
                    if vt and h == 0:
                        # rows 1:65 aren't a legal engine partition slice
                        # (starts must be 32-aligned) and DMA can't read
                        # PSUM: drain rows 0:65 to SBUF, then a SBUF->SBUF
                        # DMA shifts rows 1:65 into place.
                        araw = smp.tile([65, 512], F32, tag="araw",
                                        name="araw")
                        nc.vector.tensor_copy(out=araw, in_=avs[h][0:65, :])
                        nc.sync.dma_start(out=avsb, in_=araw[1:65, :])
                    elif pe_norm:
                        # tail AU: ACT is done with exps — drain there so
                        # the DVE queue only holds recips and muls
                        nc.scalar.copy(out=avsb, in_=avs[h][vr, :])
                    else:
                        nc.vector.tensor_copy(out=avsb, in_=avs[h][vr, :])
                    avsbs.append(avsb)
                    if pe_norm:
                        # tail-only: K=1 PE matmul broadcast — PE is idle
                        # waiting on this chain anyway and the latency is
                        # ~1/3 of the DMA bounce
                        bcs = psum.tile([64, 512], F32, tag="mm", name="rcb")
                        nc.tensor.matmul(
                            bcs, lhsT=ones64, rhs=rc, start=True, stop=True
                        )
                        bcss.append(bcs)
                        continue
                    bcs = smp.tile([64, 512], F32R, tag="bcs", name="bcs")
                    if pbcast == "gpsimd":
                        nc.gpsimd.partition_broadcast(bcs, rc, channels=64)
                    else:  # "bounce"
                        scr = drp.tile([1, 512], F32R, tag="scr", name="scr")
                        nc.sync.dma_start(out=scr, in_=rc)
                        bsrc = bass.AP(
                            tensor=scr.tensor,
                            offset=scr.offset,
                            ap=[[0, 64], [1, 512]],
                        )
                        nc.sync.dma_start(out=bcs, in_=bsrc)
                    bcss.append(bcs)
                for h in range(HPC):
                    hr = slice(h * 64, (h + 1) * 64)
                    with nc.allow_low_precision(reason="f32r attn out"):
                        nc.vector.tensor_mul(
                            out=aot[b][hr, qc * 512 : (qc + 1) * 512],
                            in0=avsbs[h],
                            in1=bcss[h],
                        )
                    if not zero_bias:
                        with nc.allow_low_precision(reason="v bias"):
                            nc.vector.tensor_scalar(
                                out=aot[b][hr, qc * 512 : (qc + 1) * 512],
                                in0=aot[b][hr, qc * 512 : (qc + 1) * 512],
                                scalar1=bias_sb[hr, 2:3],
                                scalar2=None,
                                op0=ADD,
                            )

            if probe == "attn":
                # attribution probe: attention only, inputs zeroed
                nc.vector.memset(qT, 0.0)
                if k64 == "pad":
                    nc.vector.memset(kT[0:64, 0, :], 0.0)
                    nc.vector.memset(kT[64:128, 1, :], 0.0)
                else:
                    nc.vector.memset(kT, 0.0)
                nc.vector.memset(v3[:, :, 0:64], 0.0)
                nc.vector.memset(v3[:, :, 65:129], 0.0)
                emit_small_consts()
                for r in range(reps):
                    for b in range(B):
                        for qc in range(4):
                            emit_attn(b, qc, [])
            elif probe == "qkv":
                # attribution probe: QKV strips only, no attention/store
                for r in range(reps):
                    if r == 0:
                        load_strip_part(0, 0, 4)
                        wqk_rest()
                        wv_load()
                        load_strip_part(0, 4, 8)
                        load_strip(1)
                        emit_small_consts()
                    else:
                        load_strip(r * 8)
                        load_strip(r * 8 + 1)
                    for t in range(8):
                        if t + 2 <= 7:
                            load_strip(r * 8 + t + 2)
                        for op in strip_closures(r * 8 + t):
                            op()
            for r in range(0 if probe else reps):
                base = r * 8
                if r == 0:
                    # startup order: wqk chunk 0 is already queued; stage
                    # the first x half before the bulk weight loads so the
                    # q-chain can start ~4 us earlier.
                    load_strip_part(0, 0, 4)
                    wqk_rest()
                    wv_load()
                    load_strip_part(0, 4, 8)
                    load_strip(base + 1)
                    emit_small_consts()
                else:
                    load_strip(base + 0)
                    load_strip(base + 1)
                if not early:
                    for t in range(4):
                        if t + 2 <= 7:
                            load_strip(base + t + 2)
                        for op in strip_closures(base + t):
                            op()
                    if r == 0:
                        nc.sync.dma_start(out=wp_sb, in_=wp[:, :])
                    for qc in range(4):
                        if qc + 6 <= 7:
                            load_strip(base + qc + 6)
                        emit_attn(0, qc, strip_closures(base + 4 + qc))
                else:
                    # phase 1: strips 0,1 only (ACT idle halved); strips
                    # 2,3 k/v become AU(0,0) fillers so exp starts two
                    # strips earlier. q2/q3 ride in AU(0,1).
                    load_strip(base + 2)
                    for op in strip_closures(base + 0):
                        op()
                    load_strip(base + 3)
                    for op in strip_closures(base + 1):
                        op()
                    q2, k2, v2 = strip_parts(base + 2)
                    q3, k3, v3o = strip_parts(base + 3)
                    load_strip(base + 4)
                    emit_attn(0, 0, k2 + v2 + k3 + v3o, front=8,
                              av_last=True)
                    if r == 0:
                        nc.sync.dma_start(out=wp_sb, in_=wp[:, :])
                    q4, k4, v4 = strip_parts(base + 4)
                    load_strip(base + 5)
                    emit_attn(0, 1, q2 + q3 + k4 + v4)
                    q5, k5, v5 = strip_parts(base + 5)
                    load_strip(base + 6)
                    emit_attn(0, 2, q4 + k5 + v5)
                    q6, k6, v6 = strip_parts(base + 6)
                    load_strip(base + 7)
                    emit_attn(0, 3, q5 + k6 + v6)
                # phase 3: batch-1 attention; remaining projection ops
                # spread so per-chunk PE work stays at or under the exp
                # rate. Splits stay on even (pair) bounds.
                p00, p01, p02, p03 = (proj_closures(0, i) for i in range(4))
                p10, p11, p12 = (proj_closures(1, i) for i in range(3))
                if early:
                    q7, k7, v7 = strip_parts(base + 7)
                    fl = [
                        q6 + k7 + v7,               # AU(1,0)
                        q7 + p00 + p01,             # AU(1,1)
                        p02 + p03 + p10[:2],        # AU(1,2)
                        p10[2:] + p11 + p12[:4],    # AU(1,3)
                    ]
                    tail = p12[4:] + proj_closures(1, 3, split_store=True)
                    for qc in range(4):
                        emit_attn(1, qc, fl[qc],
                                  front=8 if qc == 0 else 0,
                                  av_last=(qc == 0),
                                  hold=8 if qc == 3 else 0,
                                  pe_norm=(qc == 3))
                    for op in tail:
                        op()
                else:
                    fl = [
                        p00 + p01[:6],              # AU(1,0)
                        p01[6:] + p02 + p10[:4],    # AU(1,1)
                        p10[4:] + p03 + p11[:2],    # AU(1,2)
                        p11[2:] + p12,              # AU(1,3)
                    ]
                    for qc in range(4):
                        emit_attn(1, qc, fl[qc], hold=8 if qc == 3 else 0,
                                  pe_norm=(qc == 3))
                    for op in proj_closures(1, 3, split_store=True):
                        op()
    nc.compile()
    _NC_CACHE[key] = nc
    return nc


def prep_inputs(x, Wqkv, bqkv, Wproj):
    """Host-side sharding: returns the per-core input maps."""
    import ml_dtypes

    BF = ml_dtypes.bfloat16
    x = np.asarray(x, dtype=np.float32)
    Wqkv = np.asarray(Wqkv, dtype=np.float32)
    bqkv = np.asarray(bqkv, dtype=np.float32)
    Wproj = np.asarray(Wproj, dtype=np.float32)

    xT = np.ascontiguousarray(x.reshape(T, D).T).astype(BF)
    qk_scale = np.float32(HD ** -0.25)
    cst = np.ones((1, 64), dtype=np.float32)

    in_maps = []
    for c in range(N_CORES):
        f0 = c * HPC * HD  # first local feature column
        wqk_c = np.concatenate(
            [Wqkv[:, f0 : f0 + 128], Wqkv[:, D + f0 : D + f0 + 128]], axis=1
        ) * qk_scale
        wv_c = Wqkv[:, 2 * D + f0 : 2 * D + f0 + 128]
        wp_c = Wproj[f0 : f0 + 128, :]
        bias_c = np.stack(
            [
                bqkv[f0 : f0 + 128] * qk_scale,
                bqkv[D + f0 : D + f0 + 128] * qk_scale,
                bqkv[2 * D + f0 : 2 * D + f0 + 128],
            ],
            axis=1,
        )
        in_maps.append(
            {
                "xT": xT,
                "wqk": np.ascontiguousarray(wqk_c).astype(BF),
                "wv": np.ascontiguousarray(wv_c).astype(BF),
                "wp": np.ascontiguousarray(wp_c).astype(BF),
                "bias": np.ascontiguousarray(bias_c),
                "cst": cst,
            }
        )
    return in_maps


def kernel(x, Wqkv, bqkv, Wproj, bproj, **kw):
    from concourse.bass_utils import run_bass_kernel_spmd

    zb = not np.asarray(bqkv).any()
    nc = build_nc(zero_bias=zb, **kw)
    in_maps = prep_inputs(x, Wqkv, bqkv, Wproj)
    res = run_bass_kernel_spmd(nc, in_maps, core_ids=list(range(N_CORES)))
    total = res.results[0]["out"].astype(np.float32).copy()
    for c in range(1, N_CORES):
        total += res.results[c]["out"]
    total += np.asarray(bproj, dtype=np.float32)[None, :]
    return total.reshape(B, S, D)


if __name__ == "__main__":
    rng = np.random.default_rng(0)
    x = rng.standard_normal((B, S, D)).astype(np.float32)
    Wqkv = (rng.standard_normal((D, 3 * D)) * D**-0.5).astype(np.float32)
    bqkv = np.zeros(3 * D, np.float32)
    Wproj = (rng.standard_normal((D, D)) * D**-0.5).astype(np.float32)
    bproj = np.zeros(D, np.float32)
    got = kernel(x, Wqkv, bqkv, Wproj, bproj)
    print("ran ok", got.shape, got.dtype)

